# revision 6
# baseline (speedup 1.0000x reference)
"""TRN2 Bass kernel for PerceiverAttention (GQA cross-attention, 8-core data parallel).

Per-core computation (batch b on core b), all matmuls bf16 with fp32 PSUM:
  hs = concat(context, latents)            # [KV, H], KV = S + NL
  Q  = latents @ Wq                        # [NL, NH*DK]
  K  = hs @ Wk, V = hs @ Wv                # [KV, KVH*DK]
  per kv-head group: S^T = K_h Q_g^T (PE), attn = exp(S^T/sqrt(DK)) (ACT; no
  max-subtraction - scores are N(0,1) scale), out = attnT.T @ [V|1] (PE, ones
  column yields softmax sums), normalize by reciprocal sums, out-proj
  accumulated per head into OUT.

Hardware rules established by probing:
  - every matmul operand/output must have base partition 0
  - 2-byte matmuls only (4-byte matmuls hit the 1-sync-wait LDW limit)
  - psum tiles are bank-granular (8 x [128 x 2KB]); tags multiply pool slots

Emission order (drives Tile priorities + DMA order): latents/LT -> Wk/Wv ->
context K/V loop -> Q projection -> attention/out-proj. DMA is ~610us vs PE
~690us per core, so compute-bound when overlapped; Q lands before stage-2 PE
drains, Wo streams during attention.
"""
import math
from contextlib import ExitStack
from dataclasses import dataclass

import numpy as np

import concourse.tile as tile
from concourse import bacc, mybir
from concourse.bass_utils import run_bass_kernel_spmd
from concourse.masks import make_identity

F32 = mybir.dt.float32
CDT = mybir.dt.float16   # compute dtype: fp16 = same PE speed as bf16, 8x mantissa
EXP = mybir.ActivationFunctionType.Exp
P = 128


@dataclass(frozen=True)
class Cfg:
    H: int = 4096      # hidden
    NH: int = 32       # query heads
    DK: int = 128      # head dim
    KVH: int = 4       # kv heads
    NL: int = 64       # latents (q_len)
    S: int = 4096      # context len
    n_cores: int = 8

    @property
    def G(self):
        return self.NH // self.KVH

    @property
    def KV(self):
        return self.S + self.NL

    @property
    def HC(self):
        return self.H // P           # hidden chunks

    @property
    def SC(self):
        return self.S // P           # full context kv chunks (tail = latents)

    @property
    def NKV(self):
        return self.SC + 1           # kv chunks incl. latents tail

    @property
    def DQ(self):
        return self.NH * self.DK     # q projection width

    @property
    def DKV(self):
        return self.KVH * self.DK    # kv projection width

    @property
    def NB(self):
        return self.DQ // 512        # out-proj col blocks

    @property
    def GW(self):
        return self.G * self.NL      # score cols per kv-head group

    @property
    def NPAIR(self):
        return self.G // 2           # head pairs per group


FULL = Cfg()


def build_bass(cfg: Cfg, dbg: bool = False):
    c = cfg
    assert c.NL == 64 and c.DK == 128 and c.H % P == 0 and c.S % P == 0
    assert c.DQ % 512 == 0 and c.G % 2 == 0
    scale = 1.0 / math.sqrt(c.DK)
    VW = c.DK + 1                      # V tile width with ones column
    QHW = min(c.DQ, 2048)              # Q-projection half width
    NQH = c.DQ // QHW                  # number of Q passes
    QHB = QHW // 512                   # psum blocks per Q pass

    nc = bacc.Bacc("TRN2", target_bir_lowering=False, debug=False,
                   num_devices=c.n_cores)
    latents = nc.dram_tensor("latents", [c.NL, c.H], F32, kind="ExternalInput").ap()
    context = nc.dram_tensor("context", [c.S, c.H], F32, kind="ExternalInput").ap()
    wq = nc.dram_tensor("Wq", [c.H, c.DQ], F32, kind="ExternalInput").ap()
    wk = nc.dram_tensor("Wk", [c.H, c.DKV], F32, kind="ExternalInput").ap()
    wv = nc.dram_tensor("Wv", [c.H, c.DKV], F32, kind="ExternalInput").ap()
    wo = nc.dram_tensor("Wo", [c.DQ, c.H], F32, kind="ExternalInput").ap()
    out_d = nc.dram_tensor("out", [c.NL, c.H], F32, kind="ExternalOutput").ap()
    if dbg:
        dbg_q = nc.dram_tensor("dbg_q", [c.NL, c.DQ], F32, kind="ExternalOutput").ap()
        dbg_lt = nc.dram_tensor("dbg_lt", [P, c.HC * c.NL], F32,
                                kind="ExternalOutput").ap()
        dbg_kt0 = nc.dram_tensor("dbg_kt0", [P, c.KV], F32,
                                 kind="ExternalOutput").ap()
        dbg_v0 = nc.dram_tensor("dbg_v0", [P, (c.SC + 1) * (c.DK + 1)], F32,
                                kind="ExternalOutput").ap()
        dbg_at = nc.dram_tensor("dbg_at", [P, c.GW], F32, kind="ExternalOutput").ap()
        dbg_pa = nc.dram_tensor("dbg_pa", [P, (c.G // 2) * (c.DK + 1)], F32,
                                kind="ExternalOutput").ap()

    with tile.TileContext(nc) as tc, ExitStack() as top:
        const = top.enter_context(tc.tile_pool(name="const", bufs=1))
        ident = const.tile([P, P], CDT)
        make_identity(nc, ident[:])

        # resident tensors
        res = top.enter_context(tc.tile_pool(name="res", bufs=1))
        lt = res.tile([P, c.HC * c.NL], CDT, name="lt")        # latents^T
        qt = res.tile([P, c.NH * c.NL], CDT, name="qt")        # Q^T per head
        kt = [res.tile([P, c.KV], CDT, name=f"kt{h}") for h in range(c.KVH)]
        vres = [res.tile([P, c.NKV * VW], CDT, name=f"v{h}") for h in range(c.KVH)]

        # shared psum pool for all PE transposes (single tag -> 2 banks)
        tp_ps = top.enter_context(tc.tile_pool(name="tp_ps", bufs=2, space="PSUM"))

        def tp_tile():
            return tp_ps.tile([P, P], CDT, tag="tp", name="tp")

        dbg_pool = top.enter_context(tc.tile_pool(name="dbgp", bufs=2)) if dbg \
            else None

        def dump(dram_ap, src_ap, shape):
            p_, n_ = shape
            for col in range(0, n_, 512):
                w_ = min(512, n_ - col)
                t = dbg_pool.tile([P, 512], F32, tag="dbgt", name="dbgt")
                nc.vector.tensor_copy(t[0:p_, 0:w_], src_ap[:, col:col + w_])
                nc.sync.dma_start(dram_ap[:, col:col + w_], t[0:p_, 0:w_])

        # ---- latents -> LT ----
        with ExitStack() as st:
            lat_pool = st.enter_context(tc.tile_pool(name="lat", bufs=1))
            lat_f = lat_pool.tile([c.NL, c.H], F32)
            nc.sync.dma_start(lat_f[:], latents)
            lat_b = lat_pool.tile([c.NL, c.H], CDT)
            nc.vector.tensor_copy(lat_b[:], lat_f[:])
            for a in range(c.HC):
                ps = tp_tile()
                nc.tensor.transpose(ps[0:P, 0:c.NL], lat_b[:, a * P:(a + 1) * P],
                                    ident[0:c.NL, 0:c.NL])
                nc.scalar.copy(lt[:, a * c.NL:(a + 1) * c.NL], ps[0:P, 0:c.NL])

        # ---- stage 2: K/V projection over kv chunks ----
        with ExitStack() as st:
            wkv_pool = st.enter_context(tc.tile_pool(name="wkv", bufs=1))
            wk_bf = wkv_pool.tile([P, c.HC * c.DKV], CDT, name="wk_bf")
            wv_bf = wkv_pool.tile([P, c.HC * c.DKV], CDT, name="wv_bf")
            with ExitStack() as wst:
                wkv_f32 = wst.enter_context(tc.tile_pool(name="wkv_f32", bufs=4))
                for a in range(c.HC):
                    for (w_d, w_t) in ((wk, wk_bf), (wv, wv_bf)):
                        f = wkv_f32.tile([P, c.DKV], F32, tag="wkvf", name="wkvf")
                        nc.sync.dma_start(f[:], w_d[a * P:(a + 1) * P, :])
                        nc.vector.tensor_copy(w_t[:, a * c.DKV:(a + 1) * c.DKV],
                                              f[:])

            CHW = min(c.H, 2048)       # context DMA half width
            NCH = c.H // CHW
            ctx_f32 = st.enter_context(tc.tile_pool(name="ctx_f32", bufs=3))
            ctx_bf = st.enter_context(tc.tile_pool(name="ctx_bf", bufs=2 * NCH))
            hst_pool = st.enter_context(tc.tile_pool(name="hst", bufs=2))
            kv_ps = st.enter_context(tc.tile_pool(name="kv_ps", bufs=4, space="PSUM"))
            stage_pool = st.enter_context(tc.tile_pool(name="kstage", bufs=2))

            for kc in range(c.NKV):
                tail = kc == c.SC
                kvn = c.NL if tail else P
                if not tail:
                    cbs = []
                    for ch in range(NCH):
                        cf = ctx_f32.tile([P, CHW], F32, tag="ctxf", name="ctxf")
                        nc.sync.dma_start(
                            cf[:],
                            context[kc * P:(kc + 1) * P, ch * CHW:(ch + 1) * CHW])
                        cb = ctx_bf.tile([P, CHW], CDT, tag="ctxb", name="ctxb")
                        nc.gpsimd.tensor_copy(cb[:], cf[:])
                        cbs.append(cb)
                    hst = hst_pool.tile([P, c.H], CDT, tag="hst", name="hst")
                    apc = CHW // P
                    for a in range(c.HC):
                        ps = tp_tile()
                        nc.tensor.transpose(
                            ps[:],
                            cbs[a // apc][:, (a % apc) * P:(a % apc + 1) * P],
                            ident[:])
                        if a % 2 == 0:
                            nc.vector.tensor_copy(hst[:, a * P:(a + 1) * P], ps[:])
                        else:
                            nc.scalar.copy(hst[:, a * P:(a + 1) * P], ps[:])

                    def lhs(a, _h=hst):
                        return _h[:, a * P:(a + 1) * P]
                else:
                    def lhs(a):
                        return lt[:, a * c.NL:(a + 1) * c.NL]

                kps = kv_ps.tile([P, c.DKV], F32, tag="kvps", name="kps")
                for a in range(c.HC):
                    nc.tensor.matmul(kps[0:kvn, :], lhs(a),
                                     wk_bf[:, a * c.DKV:(a + 1) * c.DKV],
                                     start=(a == 0), stop=(a == c.HC - 1))
                vps = kv_ps.tile([P, c.DKV], F32, tag="kvps", name="vps")
                for a in range(c.HC):
                    nc.tensor.matmul(vps[0:kvn, :], lhs(a),
                                     wv_bf[:, a * c.DKV:(a + 1) * c.DKV],
                                     start=(a == 0), stop=(a == c.HC - 1))

                k_bf = stage_pool.tile([P, c.DKV], CDT, tag="k_bf", name="k_bf")
                nc.scalar.copy(k_bf[0:kvn, :], kps[0:kvn, :])
                for h in range(c.KVH):
                    ps = tp_tile()
                    nc.tensor.transpose(ps[0:c.DK, 0:kvn],
                                        k_bf[0:kvn, h * c.DK:(h + 1) * c.DK],
                                        ident[0:kvn, 0:kvn])
                    nc.vector.tensor_copy(kt[h][:, kc * P:kc * P + kvn],
                                          ps[0:c.DK, 0:kvn])
                    nc.vector.tensor_copy(
                        vres[h][0:kvn, kc * VW:kc * VW + c.DK],
                        vps[0:kvn, h * c.DK:(h + 1) * c.DK])
                    nc.vector.memset(vres[h][0:kvn, kc * VW + c.DK:(kc + 1) * VW],
                                     1.0)

        if dbg:
            dump(dbg_kt0, kt[0][:], [P, c.KV])
            dump(dbg_v0, vres[0][:], [P, c.NKV * VW])

        # ---- Q projection (after K/V so attention-critical DMA comes later) ----
        with ExitStack() as st:
            wq_ps = st.enter_context(tc.tile_pool(name="wq_ps", bufs=QHB,
                                                  space="PSUM"))
            wq_f32 = st.enter_context(tc.tile_pool(name="wq_f32", bufs=2))
            wq_bfp = st.enter_context(tc.tile_pool(name="wq_bf", bufs=2))
            qsb = st.enter_context(tc.tile_pool(name="qsb", bufs=1))
            q_bf = qsb.tile([c.NL, c.DQ], CDT, name="q_bf")
            for qh in range(NQH):
                qps = [wq_ps.tile([c.NL, 512], F32, tag="qps", name="qps")
                       for _ in range(QHB)]
                for a in range(c.HC):
                    wq_f = wq_f32.tile([P, QHW], F32, tag="wqf", name="wqf")
                    nc.sync.dma_start(
                        wq_f[:], wq[a * P:(a + 1) * P, qh * QHW:(qh + 1) * QHW])
                    wq_b = wq_bfp.tile([P, QHW], CDT, tag="wqb", name="wqb")
                    if a % 2 == 0:
                        nc.gpsimd.tensor_copy(wq_b[:], wq_f[:])
                    else:
                        nc.vector.tensor_copy(wq_b[:], wq_f[:])
                    for b in range(QHB):
                        nc.tensor.matmul(qps[b][:], lt[:, a * c.NL:(a + 1) * c.NL],
                                         wq_b[:, b * 512:(b + 1) * 512],
                                         start=(a == 0), stop=(a == c.HC - 1))
                for b in range(QHB):
                    nc.scalar.copy(
                        q_bf[:, qh * QHW + b * 512:qh * QHW + (b + 1) * 512],
                        qps[b][:])
            for h in range(c.NH):
                ps = tp_tile()
                nc.tensor.transpose(ps[0:P, 0:c.NL], q_bf[:, h * P:(h + 1) * P],
                                    ident[0:c.NL, 0:c.NL])
                nc.scalar.copy(qt[:, h * c.NL:(h + 1) * c.NL], ps[0:P, 0:c.NL])
            if dbg:
                dump(dbg_q, q_bf[:], [c.NL, c.DQ])
                dump(dbg_lt, lt[:], [P, c.HC * c.NL])

        # ---- stage 3: attention + out-proj per kv-head group ----
        with ExitStack() as st:
            WHW = min(c.H, 2048)       # Wo DMA half width
            NWH = c.H // WHW
            wo_f32 = st.enter_context(tc.tile_pool(name="wo_f32", bufs=3))
            wo_bfp = st.enter_context(tc.tile_pool(name="wo_bf", bufs=c.G + 1))
            osb = st.enter_context(tc.tile_pool(name="osb", bufs=1))
            out_sb = osb.tile([c.NL, c.H], F32, name="out_sb")
            st_ps = st.enter_context(tc.tile_pool(name="st_ps", bufs=2, space="PSUM"))
            pa_ps = st.enter_context(tc.tile_pool(name="pa_ps", bufs=1, space="PSUM"))
            po_ps = st.enter_context(tc.tile_pool(name="po_ps", bufs=2, space="PSUM"))
            at_pool = st.enter_context(tc.tile_pool(name="at", bufs=3))
            nrm_pool = st.enter_context(tc.tile_pool(name="nrm", bufs=4))

            for hk in range(c.KVH):
                wo_tiles = []
                for g in range(c.G):
                    head = hk * c.G + g
                    wb = wo_bfp.tile([P, c.H], CDT, tag="wob", name="wob")
                    for wh in range(NWH):
                        wf = wo_f32.tile([P, WHW], F32, tag="wof", name="wof")
                        nc.sync.dma_start(
                            wf[:],
                            wo[head * P:(head + 1) * P, wh * WHW:(wh + 1) * WHW])
                        dst = wb[:, wh * WHW:(wh + 1) * WHW]
                        if (g + wh) % 2 == 0:
                            nc.gpsimd.tensor_copy(dst, wf[:])
                        else:
                            nc.vector.tensor_copy(dst, wf[:])
                    wo_tiles.append(wb)

                # pair accumulators: NPAIR slots of [128, VW], 2 per bank tile
                n_pa = (c.NPAIR + 1) // 2
                pas = [pa_ps.tile([P, min(2, c.NPAIR - 2 * i) * VW], F32,
                                  name=f"pa{i}", tag=f"pa{i}")
                       for i in range(n_pa)]

                def pa_slot(j, _pas=pas):
                    return _pas[j // 2][:, (j % 2) * VW:(j % 2) * VW + VW]

                for kc in range(c.NKV):
                    tail = kc == c.SC
                    kvn = c.NL if tail else P
                    sps = st_ps.tile([P, c.GW], F32, tag="sps", name="sps")
                    nc.tensor.matmul(sps[0:kvn, :], kt[hk][:, kc * P:kc * P + kvn],
                                     qt[:, hk * c.GW:(hk + 1) * c.GW],
                                     start=True, stop=True)
                    at = at_pool.tile([P, c.GW], CDT, tag="at", name="at")
                    nc.scalar.activation(at[0:kvn, :], sps[0:kvn, :], EXP,
                                         scale=scale)
                    if dbg and hk == 0 and kc == 0:
                        dump(dbg_at, at[:], [P, c.GW])
                    for j in range(c.NPAIR):
                        # start=True clears the ENTIRE psum bank's has_written
                        # bits (HW-verified), so only the bank's first slot may
                        # issue it; the second slot's first write lands on
                        # cleared bits and overwrites, then accumulates.
                        nc.tensor.matmul(
                            pa_slot(j),
                            at[0:kvn, j * 2 * c.NL:(j + 1) * 2 * c.NL],
                            vres[hk][0:kvn, kc * VW:(kc + 1) * VW],
                            start=(kc == 0 and j % 2 == 0),
                            stop=(kc == c.NKV - 1))

                if dbg and hk == 0:
                    for j in range(c.NPAIR):
                        dump(dbg_pa[:, j * VW:(j + 1) * VW], pa_slot(j), [P, VW])

                # normalize + transpose -> pairT, then out-proj
                pair_ts = []
                for j in range(c.NPAIR):
                    sl = pa_slot(j)
                    sums = nrm_pool.tile([P, 1], F32, tag="sums", name="sums")
                    nc.vector.tensor_copy(sums[:], sl[:, c.DK:c.DK + 1])
                    rcp = nrm_pool.tile([P, 1], F32, tag="rcp", name="rcp")
                    nc.vector.reciprocal(rcp[:], sums[:])
                    pn = nrm_pool.tile([P, c.DK], CDT, tag="pn", name="pn")
                    nc.vector.tensor_scalar_mul(pn[:], sl[:, 0:c.DK], rcp[:])
                    ps = tp_tile()
                    nc.tensor.transpose(ps[:], pn[:], ident[:])
                    ptile = nrm_pool.tile([P, P], CDT, tag="pairt", name="pairt")
                    nc.scalar.copy(ptile[:], ps[:])
                    pair_ts.append(ptile)

                for b in range(c.NB):
                    pob = po_ps.tile([c.NL, 512], F32, tag="pob", name="pob")
                    n_mm = 2 * c.NPAIR
                    for j in range(c.NPAIR):
                        for s_ in range(2):
                            i_mm = j * 2 + s_
                            nc.tensor.matmul(
                                pob[:],
                                pair_ts[j][:, s_ * c.NL:(s_ + 1) * c.NL],
                                wo_tiles[i_mm][:, b * 512:(b + 1) * 512],
                                start=(i_mm == 0), stop=(i_mm == n_mm - 1))
                    dst = out_sb[:, b * 512:(b + 1) * 512]
                    if hk == 0:
                        nc.vector.tensor_copy(dst, pob[:])
                    else:
                        nc.vector.tensor_add(dst, dst, pob[:])

            nc.sync.dma_start(out_d, out_sb[:])

    nc.compile()
    return nc


_CACHE = {}


def _get_nc(cfg: Cfg):
    if cfg not in _CACHE:
        _CACHE[cfg] = build_bass(cfg)
    return _CACHE[cfg]


def run(inputs: dict, cfg: Cfg, trace: bool = False):
    c = cfg
    lat = np.asarray(inputs["latents"], dtype=np.float32)
    ctx = np.asarray(inputs["context"], dtype=np.float32)
    wq_a = np.ascontiguousarray(np.asarray(inputs["Wq"], dtype=np.float32))
    wk_a = np.ascontiguousarray(np.asarray(inputs["Wk"], dtype=np.float32))
    wv_a = np.ascontiguousarray(np.asarray(inputs["Wv"], dtype=np.float32))
    wo_a = np.ascontiguousarray(np.asarray(inputs["Wo"], dtype=np.float32))
    B = lat.shape[0]
    assert B == c.n_cores
    nc = _get_nc(c)
    in_maps = [{
        "latents": np.ascontiguousarray(lat[b]),
        "context": np.ascontiguousarray(ctx[b]),
        "Wq": wq_a, "Wk": wk_a, "Wv": wv_a, "Wo": wo_a,
    } for b in range(B)]
    r = run_bass_kernel_spmd(nc, in_maps, list(range(B)), trace=trace)
    out = np.stack([r.results[b]["out"] for b in range(B)], axis=0)
    return out, r


def kernel(latents, context, Wq, Wk, Wv, Wo):
    out, _ = run(dict(latents=latents, context=context, Wq=Wq, Wk=Wk,
                      Wv=Wv, Wo=Wo), FULL)
    return out
